# revision 7
# baseline (speedup 1.0000x reference)
"""Trainium2 Bass kernel for nn_ChannelAttGatedGRUCell.

Reference computation (per batch element b):
    xh = concat([x, h], -1)                                  # (C, 2048)
    r = attn(xh; Wq_r, Wk_r, Wv_r); z = attn(xh; ...z)       # (C, 1024)
    reset = sigmoid(r); update = sigmoid(z)
    xhr = concat([x, h*reset], -1)
    n = attn(xhr; ...n)
    new = (1-update)*h + update*tanh(n)
    out = LayerNorm(new) * gamma + beta

Sharding: data-parallel over batch B=64 across 8 cores (8 per core);
weights replicated.  The host pre-transposes x/h to feature-major
(xT/hT) so the kernel needs no on-device transposes (fp32 has no DMA
transpose path).  All matmuls run as float32r (fp32 truncated to FP22,
1 cycle/row at N=512 - bf16 speed, ~13-bit mantissa).

On-device dataflow per batch element (all layouts chosen so every
matmul contracts over the partition dim with no transposes):
    Q_T[d,c]  = Wq[k,d].T @ xhT[k,c]       (lhsT=Wq chunk, rhs=xhT)
    K_T[d,c]  = Wk[k,d].T @ xhT[k,c]
    V[e,d]    = xhT[k,e].T @ Wv[k,d]       (lhsT=xhT chunk, rhs=Wv)
    S_T[e,c]  = K_T[d,e].T @ Q_T[d,c]      (scores transposed)
    E_T[e,c]  = exp(S_T * 1/sqrt(dqk))     (ScalarE, straight from PSUM)
    s[c]      = ones.T @ E_T               (softmax denom via matmul)
    U[c,d]    = E_T[e,c].T @ V[e,d]        (z/n gates; c on partitions)
    U_rT[d,c] = V[e,d].T @ E_T[e,c]        (r gate; d on partitions, so
                                            h_T*sigmoid(U_rT) directly
                                            yields the transposed xhr)
    softmax 1/s folds into the ScalarE sigmoid/tanh `scale` operand
    (z/n) or into one DVE rescale of E_T (r gate).

Gate order is (z, r, n): after z no longer needs h_T, the r gate's
reset multiply overwrites h_T in place with (h*reset)_T, which the n
gate then reads as the second half of xhr_T.  The z update gate is
spilled to a DRAM scratch buffer and re-read during the final gating
to stay inside SBUF.  Batch elements are processed in pairs so each
weight matrix is DMA'd once per pair (192 MB of weight traffic per
core, overlapped under ~1.6 ms of PE work).
"""

import numpy as np

B, C, D_IN, D_H, D_QK = 64, 512, 1024, 1024, 512
DIN2 = D_IN + D_H
N_CORES = 8
NB = B // N_CORES          # batch elements per core
EPS = 1e-5
P = 128
KC = DIN2 // P             # 16 contraction chunks over concat features
KX = D_IN // P             # 8  (x-half chunks; h-half is chunks KX..KC-1)
CT = C // P                # 4  c-tiles (tokens)
ET = CT                    # 4  e-tiles (tokens, attended-over axis)
DTQ = D_QK // P            # 4  d-tiles of q/k head dim
DTH = D_H // P             # 8  d-tiles of value dim
FD = 512                   # matmul moving free dim (fp32 PSUM bank)
NDH = D_H // FD            # 2  d-halves of value dim
SM_SCALE = float(1.0 / np.sqrt(D_QK))

_CACHE = {}


def _build(nb, use_bias, use_gb):
    """Build + compile the per-core Bass program for nb batch elements."""
    import contextlib

    import concourse.bacc as bacc
    import concourse.bass as bass
    import concourse.mybir as mybir
    import concourse.tile as tile

    f32 = mybir.dt.float32
    f32r = mybir.dt.float32r
    Alu = mybir.AluOpType
    Act = mybir.ActivationFunctionType

    nc = bacc.Bacc("TRN2", target_bir_lowering=False, debug=False)

    # ---- DRAM I/O ----
    xT_t = nc.dram_tensor("xT", [nb, D_IN, C], f32r, kind="ExternalInput")
    hT_t = nc.dram_tensor("hT", [nb, D_H, C], f32r, kind="ExternalInput")
    hN_t = nc.dram_tensor("hN", [nb, C, D_H], f32, kind="ExternalInput")
    w_t = {}
    b_t = {}
    for g in ("r", "z", "n"):
        w_t[g, "q"] = nc.dram_tensor(f"Wq_{g}", [DIN2, D_QK], f32r, kind="ExternalInput")
        w_t[g, "k"] = nc.dram_tensor(f"Wk_{g}", [DIN2, D_QK], f32r, kind="ExternalInput")
        w_t[g, "v"] = nc.dram_tensor(f"Wv_{g}", [DIN2, D_H], f32r, kind="ExternalInput")
        b_t[g, "q"] = nc.dram_tensor(f"bq_{g}", [D_QK], f32, kind="ExternalInput")
        b_t[g, "k"] = nc.dram_tensor(f"bk_{g}", [D_QK], f32, kind="ExternalInput")
        b_t[g, "v"] = nc.dram_tensor(f"bv_{g}", [D_H], f32, kind="ExternalInput")
    gamma_t = nc.dram_tensor("gamma", [D_H], f32, kind="ExternalInput")
    beta_t = nc.dram_tensor("beta", [D_H], f32, kind="ExternalInput")
    out_t = nc.dram_tensor("out", [nb, C, D_H], f32, kind="ExternalOutput")
    u_t = nc.dram_tensor("u_scratch", [nb, C, D_H], f32)

    xT_ap = xT_t.ap()
    hT_ap = hT_t.ap()
    hN_ap = hN_t.ap()
    out_ap = out_t.ap()
    u_ap = u_t.ap()
    # feature-major [128, ko, free] views of weights
    wv_view = {k: v.ap().rearrange("(ko p) d -> p ko d", p=P) for k, v in w_t.items()}

    with tile.TileContext(nc) as tc, contextlib.ExitStack() as ctx:
        consts = ctx.enter_context(tc.tile_pool(name="consts", bufs=1))
        wpool = ctx.enter_context(tc.tile_pool(name="wpool", bufs=2))
        apool = ctx.enter_context(tc.tile_pool(name="apool", bufs=2))
        tmp5 = ctx.enter_context(tc.tile_pool(name="tmp5", bufs=4))
        tmp10 = ctx.enter_context(tc.tile_pool(name="tmp10", bufs=3))
        newp = ctx.enter_context(tc.tile_pool(name="newp", bufs=2))
        stat = ctx.enter_context(tc.tile_pool(name="stat", bufs=6))
        pspool = ctx.enter_context(tc.tile_pool(name="pspool", bufs=8, space="PSUM"))

        # fp32r matmuls need even innermost free counts, so the ones
        # vector used for softmax denominators is two identical columns.
        ones_f = consts.tile([P, 2], f32, name="ones_f", tag="ones_f")
        nc.vector.memset(ones_f, 1.0)
        ones = consts.tile([P, 2], f32r, name="ones", tag="ones")
        nc.vector.tensor_copy(ones, ones_f)
        eps_sb = consts.tile([P, 1], f32, name="eps_sb", tag="eps_sb")
        nc.vector.memset(eps_sb, EPS)

        bias_col = {}
        bv_bc = {}
        if use_bias:
            for g in ("r", "z", "n"):
                for m in ("q", "k"):
                    t = consts.tile([P, DTQ], f32, name=f"b{m}{g}", tag=f"b{m}{g}")
                    nc.sync.dma_start(
                        t, b_t[g, m].ap().rearrange("(dt p) -> p dt", p=P)
                    )
                    bias_col[g, m] = t
                t = consts.tile([P, D_H], f32, name=f"bv{g}", tag=f"bv{g}")
                src = b_t[g, "v"].ap()
                nc.sync.dma_start(
                    t,
                    bass.AP(
                        tensor=src.tensor, offset=src.offset, ap=[[0, P], src.ap[0]]
                    ),
                )
                bv_bc[g] = t
        gamma_bc = beta_bc = None
        if use_gb:
            gamma_bc = consts.tile([P, D_H], f32, name="gamma_bc", tag="gamma_bc")
            beta_bc = consts.tile([P, D_H], f32, name="beta_bc", tag="beta_bc")
            for t, src_t in ((gamma_bc, gamma_t), (beta_bc, beta_t)):
                src = src_t.ap()
                nc.sync.dma_start(
                    t,
                    bass.AP(
                        tensor=src.tensor, offset=src.offset, ap=[[0, P], src.ap[0]]
                    ),
                )

        def r32(ap):
            return ap  # tiles feeding matmuls are float32r end-to-end

        assert nb % 2 == 0
        for pi in range(nb // 2):
            bpair = (2 * pi, 2 * pi + 1)
            xT_sb = {}
            hT_sb = {}  # holds h_T for gates z/r, overwritten to (h*reset)_T
            for b in bpair:
                xT_sb[b] = apool.tile([P, KX, C], f32r, name=f"xT_{b}", tag="xT")
                nc.sync.dma_start(
                    xT_sb[b], xT_ap[b].rearrange("(ko p) c -> p ko c", p=P)
                )
                hT_sb[b] = apool.tile([P, KX, C], f32r, name=f"hT_{b}", tag="hhr")
                nc.sync.dma_start(
                    hT_sb[b], hT_ap[b].rearrange("(ko p) c -> p ko c", p=P)
                )

            def xhT(b, kc):
                if kc < KX:
                    return xT_sb[b][:, kc, :]
                return hT_sb[b][:, kc - KX, :]

            for gate in ("z", "r", "n"):
                qt = {}
                kt = {}
                v = {}
                # ---- Q_T / K_T projections (weight-stationary) ----
                for b in bpair:
                    qt[b] = apool.tile([P, DTQ, C], f32r, name=f"qt_{b}", tag="qt")
                    kt[b] = apool.tile([P, DTQ, C], f32r, name=f"kt_{b}", tag="kt")
                for m, dst in (("q", qt), ("k", kt)):
                    for dt in range(DTQ):
                        w = wpool.tile(
                            [P, KC, P], f32r, name=f"w{m}{dt}_{gate}_{pi}", tag="wqk"
                        )
                        nc.sync.dma_start(
                            w, wv_view[gate, m][:, :, dt * P : (dt + 1) * P]
                        )
                        for b in bpair:
                            ps = pspool.tile(
                                [P, FD], f32, name=f"ps{m}{b}{dt}", tag="ps"
                            )
                            for kc in range(KC):
                                nc.tensor.matmul(
                                    ps,
                                    r32(w[:, kc, :]),
                                    r32(xhT(b, kc)),
                                    start=(kc == 0),
                                    stop=(kc == KC - 1),
                                )
                            if use_bias:
                                nc.vector.tensor_scalar_add(
                                    dst[b][:, dt, :],
                                    ps,
                                    bias_col[gate, m][:, dt : dt + 1],
                                )
                            else:
                                nc.vector.tensor_copy(dst[b][:, dt, :], ps)
                # ---- V projection (xhT-stationary, token-major out) ----
                for b in bpair:
                    v[b] = apool.tile([P, ET, D_H], f32r, name=f"v_{b}", tag="vv")
                for dh in range(NDH):
                    pv = {}
                    for b in bpair:
                        for e in range(ET):
                            pv[b, e] = pspool.tile(
                                [P, FD], f32, name=f"psv{b}{e}", tag="ps"
                            )
                    for kc in range(KC):
                        wv = wpool.tile(
                            [P, FD], f32r, name=f"wv{dh}{kc}_{gate}_{pi}",
                            tag="wv", bufs=4,
                        )
                        nc.sync.dma_start(
                            wv, wv_view[gate, "v"][:, kc, dh * FD : (dh + 1) * FD]
                        )
                        for b in bpair:
                            for e in range(ET):
                                nc.tensor.matmul(
                                    pv[b, e],
                                    r32(xhT(b, kc)[:, e * P : (e + 1) * P]),
                                    r32(wv),
                                    start=(kc == 0),
                                    stop=(kc == KC - 1),
                                )
                    for b in bpair:
                        for e in range(ET):
                            dstv = v[b][:, e, dh * FD : (dh + 1) * FD]
                            if use_bias:
                                nc.vector.tensor_tensor(
                                    dstv,
                                    pv[b, e],
                                    bv_bc[gate][:, dh * FD : (dh + 1) * FD],
                                    Alu.add,
                                )
                            else:
                                nc.scalar.copy(dstv, pv[b, e])

                # ---- attention ----
                for b in bpair:
                    et_sb = apool.tile([P, ET, C], f32r, name=f"et_{b}", tag="et")
                    for e in range(ET):
                        ps = pspool.tile([P, FD], f32, name=f"pss{b}{e}", tag="ps")
                        for dk in range(DTQ):
                            nc.tensor.matmul(
                                ps,
                                r32(kt[b][:, dk, e * P : (e + 1) * P]),
                                r32(qt[b][:, dk, :]),
                                start=(dk == 0),
                                stop=(dk == DTQ - 1),
                            )
                        nc.scalar.activation(
                            et_sb[:, e, :], ps, Act.Exp, scale=SM_SCALE
                        )
                    if gate == "r":
                        # softmax denom as a row vector; rescale E_T by 1/s
                        ps = pspool.tile([P, FD], f32, name=f"psr{b}", tag="ps")
                        for e in range(ET):
                            nc.tensor.matmul(
                                ps[0:2, :],
                                r32(ones),
                                r32(et_sb[:, e, :]),
                                start=(e == 0),
                                stop=(e == ET - 1),
                            )
                        rrow = tmp5.tile([P, C], f32, name=f"rrow{b}", tag="tmp5")
                        nc.vector.reciprocal(rrow[0:1, :], ps[0:1, :])
                        rbc = tmp5.tile([P, C], f32, name=f"rbc{b}", tag="tmp5")
                        nc.gpsimd.partition_broadcast(rbc, rrow[0:1, :])
                        nc.vector.tensor_tensor(
                            et_sb[:],
                            et_sb[:],
                            rbc[:, None, :].to_broadcast([P, ET, C]),
                            Alu.mult,
                        )
                        # U_rT[d,c] -> sigmoid -> hT_sb *= reset_T (in place)
                        for dt in range(DTH):
                            psr = pspool.tile(
                                [P, FD], f32, name=f"psu{b}{dt}", tag="ps"
                            )
                            for e in range(ET):
                                nc.tensor.matmul(
                                    psr,
                                    r32(v[b][:, e, dt * P : (dt + 1) * P]),
                                    r32(et_sb[:, e, :]),
                                    start=(e == 0),
                                    stop=(e == ET - 1),
                                )
                            sg = tmp5.tile([P, C], f32, name=f"sg{b}{dt}", tag="tmp5")
                            nc.scalar.activation(sg, psr, Act.Sigmoid)
                            nc.vector.tensor_tensor(
                                hT_sb[b][:, dt, :], hT_sb[b][:, dt, :], sg, Alu.mult
                            )
                    else:
                        # softmax denom as a per-partition column
                        pc = pspool.tile([P, FD], f32, name=f"psc{b}", tag="ps")
                        for ct in range(CT):
                            for e in range(ET):
                                nc.tensor.matmul(
                                    pc[:, 2 * ct : 2 * ct + 2],
                                    r32(et_sb[:, e, ct * P : (ct + 1) * P]),
                                    r32(ones),
                                    start=(e == 0),
                                    stop=(e == ET - 1),
                                )
                        # columns come in identical pairs; invert all 2*CT
                        # and index the even ones below
                        rcol = stat.tile([P, 2 * CT], f32, name=f"rcol{b}", tag="rcol")
                        nc.vector.reciprocal(rcol, pc[:, 0 : 2 * CT])
                        for ct in range(CT):
                            if gate == "n":
                                hN_ct = tmp10.tile(
                                    [P, D_H], f32, name=f"hN{b}{ct}", tag="tmp10"
                                )
                                nc.sync.dma_start(
                                    hN_ct, hN_ap[b, ct * P : (ct + 1) * P, :]
                                )
                                u_ct = tmp10.tile(
                                    [P, D_H], f32, name=f"u{b}{ct}", tag="tmp10"
                                )
                                nc.sync.dma_start(
                                    u_ct, u_ap[b, ct * P : (ct + 1) * P, :]
                                )
                                new_t = newp.tile(
                                    [P, D_H], f32, name=f"new{b}{ct}", tag="new"
                                )
                            for dh in range(NDH):
                                pu = pspool.tile(
                                    [P, FD], f32, name=f"psu{b}{ct}{dh}", tag="ps"
                                )
                                for e in range(ET):
                                    nc.tensor.matmul(
                                        pu,
                                        r32(et_sb[:, e, ct * P : (ct + 1) * P]),
                                        r32(v[b][:, e, dh * FD : (dh + 1) * FD]),
                                        start=(e == 0),
                                        stop=(e == ET - 1),
                                    )
                                if gate == "z":
                                    ut = tmp5.tile(
                                        [P, FD], f32, name=f"ut{b}{ct}{dh}", tag="tmp5"
                                    )
                                    nc.scalar.activation(
                                        ut, pu, Act.Sigmoid,
                                        scale=rcol[:, 2 * ct : 2 * ct + 1],
                                    )
                                    nc.sync.dma_start(
                                        u_ap[
                                            b,
                                            ct * P : (ct + 1) * P,
                                            dh * FD : (dh + 1) * FD,
                                        ],
                                        ut,
                                    )
                                else:  # gate == "n": fused gating
                                    tt = tmp5.tile(
                                        [P, FD], f32, name=f"tt{b}{ct}{dh}", tag="tmp5"
                                    )
                                    nc.scalar.activation(
                                        tt, pu, Act.Tanh,
                                        scale=rcol[:, 2 * ct : 2 * ct + 1],
                                    )
                                    hsl = hN_ct[:, dh * FD : (dh + 1) * FD]
                                    usl = u_ct[:, dh * FD : (dh + 1) * FD]
                                    nsl = new_t[:, dh * FD : (dh + 1) * FD]
                                    # new = h + u*(tanh(n) - h)
                                    nc.vector.tensor_tensor(tt, tt, hsl, Alu.subtract)
                                    nc.vector.tensor_tensor(tt, tt, usl, Alu.mult)
                                    nc.vector.tensor_tensor(nsl, tt, hsl, Alu.add)
                            if gate == "n":
                                # ---- LayerNorm over d per token row ----
                                stats = stat.tile(
                                    [P, 2, 6], f32, name=f"st{b}{ct}", tag="st"
                                )
                                for half in range(2):
                                    nc.vector.bn_stats(
                                        stats[:, half, :],
                                        new_t[:, half * FD : (half + 1) * FD],
                                    )
                                mv = stat.tile([P, 2], f32, name=f"mv{b}{ct}", tag="mv")
                                nc.vector.bn_aggr(mv, stats)
                                rstd = stat.tile(
                                    [P, 1], f32, name=f"rs{b}{ct}", tag="rs"
                                )
                                nc.scalar.activation(
                                    rstd, mv[:, 1:2], Act.Sqrt, bias=eps_sb
                                )
                                nc.vector.reciprocal(rstd, rstd)
                                nc.vector.tensor_scalar(
                                    new_t,
                                    new_t,
                                    mv[:, 0:1],
                                    rstd,
                                    op0=Alu.subtract,
                                    op1=Alu.mult,
                                )
                                if use_gb:
                                    nc.vector.tensor_tensor(
                                        new_t, new_t, gamma_bc, Alu.mult
                                    )
                                    nc.vector.tensor_tensor(
                                        new_t, new_t, beta_bc, Alu.add
                                    )
                                nc.sync.dma_start(
                                    out_ap[b, ct * P : (ct + 1) * P, :], new_t
                                )

    nc.compile()
    return nc


def _get_nc(nb, use_bias, use_gb):
    key = (nb, use_bias, use_gb)
    if key not in _CACHE:
        _CACHE[key] = _build(nb, use_bias, use_gb)
    return _CACHE[key]


def _make_in_maps(inputs, nb=NB, n_cores=N_CORES):
    x = np.ascontiguousarray(np.asarray(inputs["x"], dtype=np.float32))
    h = np.ascontiguousarray(np.asarray(inputs["h"], dtype=np.float32))
    shared = {}
    for g in ("r", "z", "n"):
        for nm in ("Wq", "Wk", "Wv", "bq", "bk", "bv"):
            shared[f"{nm}_{g}"] = np.ascontiguousarray(
                np.asarray(inputs[f"{nm}_{g}"], dtype=np.float32)
            )
    shared["gamma"] = np.ascontiguousarray(np.asarray(inputs["gamma"], np.float32))
    shared["beta"] = np.ascontiguousarray(np.asarray(inputs["beta"], np.float32))

    in_maps = []
    for ci in range(n_cores):
        sl = slice(ci * nb, (ci + 1) * nb)
        xs = x[sl]
        hs = h[sl]
        m = dict(shared)
        m["xT"] = np.ascontiguousarray(xs.transpose(0, 2, 1))
        m["hT"] = np.ascontiguousarray(hs.transpose(0, 2, 1))
        m["hN"] = hs
        in_maps.append(m)
    return in_maps


def _flags(inputs):
    use_bias = any(
        np.any(np.asarray(inputs[f"{nm}_{g}"]))
        for g in ("r", "z", "n")
        for nm in ("bq", "bk", "bv")
    )
    gamma = np.asarray(inputs["gamma"])
    beta = np.asarray(inputs["beta"])
    use_gb = (not np.allclose(gamma, 1.0)) or bool(np.any(beta))
    return bool(use_bias), bool(use_gb)


def kernel(**inputs):
    from concourse import bass_utils

    use_bias, use_gb = _flags(inputs)
    nc = _get_nc(NB, use_bias, use_gb)
    in_maps = _make_in_maps(inputs)
    res = bass_utils.run_bass_kernel_spmd(nc, in_maps, core_ids=list(range(N_CORES)))
    out = np.concatenate([r["out"] for r in res.results], axis=0)
    return np.ascontiguousarray(out.astype(np.float32))
